# revision 47
# baseline (speedup 1.0000x reference)
"""NanoVLM GQA attention block on 8 Trainium2 NeuronCores.

Sharding: core c = 4*b + g handles batch b (of 2) and head-group g (of 4):
8 q-heads (global 8g..8g+8) and 2 kv-heads (2g, 2g+1). Each core computes a
partial output restricted to its heads' columns of Wo; the host sums the 4
partials per batch (the tensor-parallel reduce, done host-side).

Device pipeline (per core, bf16 matmuls, fp32 psum accumulation):
  1. proj with WEIGHTS stationary -> q/k/v directly d-major [hd, t] in PSUM,
     blk-outer (k/v first) so each group's rope starts while the next runs
  2. RoPE in d-major: rotate_half = partition 32<->64 block swap done with
     SBUF->SBUF DMAs (sign folded into the bf16 sin table); 1/sqrt(64)
     folded into q's tables; q written into qZ[h] [128, t]: head h's 64
     rows at its kv head's partition range, other 64 rows zero
  3. v transposed back to t-major, one copy into v_aug [128k, 4, 64] =
     [ones | v_kv0 | ones | v_kv1] (each head's stationary slice is
     contiguous; ones put the softmax denominator l on psum rows 0:64,
     offset 0 as required by reciprocal_approx_fast)
  4. scores: sp [128k, 512q] = kTp_chunk^T @ qZ[h] slice at K=128 full-array
     mode (zero q rows kill the other kv head's contribution; no PE
     row-tiling mode switches), causal sub-ranges only; exp on ACT with
     per-partition bias = gate[h, 2vq+vk] + log-mask, writing P^T bf16;
     diagonal block gets a post-exp causal01 multiply
  5. PV: yp [128, 512q] += v_aug^T @ P^T (rows 0:64 = l, 64:128 = y)
  6. normalize: reciprocal_approx_fast(l) -> rls, DVE mul -> yT[hd, t]
  7. out-proj straight from yT: psum [128t, 512n] over 4 head-pair chunks,
     DVE evac bf16, DMA partial out [1024, 2048] on sync/scalar queues
Schedule: th0 proj -> s=0 attention with th1 proj/rope/v spliced as PE
fillers (time-balanced across pair starts) -> s=1 attention with the first
16 out-proj units spliced in -> trailing out-proj units.
"""

import os
import sys

sys.path.insert(0, "/opt/trn_rl_repo")

import numpy as np

import concourse.bacc as bacc
import concourse.mybir as mybir
import concourse.tile as tile
from concourse.bass_utils import run_bass_kernel_spmd
from concourse.masks import make_identity

F32 = mybir.dt.float32
BF16 = mybir.dt.bfloat16
NP_BF16 = mybir.dt.np(mybir.dt.bfloat16)
AF = mybir.ActivationFunctionType
ALU = mybir.AluOpType

B, T, C = 2, 1024, 2048
NH, NKV, HD = 32, 8, 64
QH, KVH = 8, 2          # per-core q-heads / kv-heads
NTB = T // 128          # 8 t-blocks
NCORES = 8
NEG = -1e30


def build_program(qtile_vq):
    """qtile_vq: per 128-token q-tile, the is_vision value (0/1), len 8."""
    nc = bacc.Bacc("TRN2", target_bir_lowering=False, debug=False,
                   num_devices=NCORES)

    xT_d = nc.dram_tensor("xT", [C, T], BF16, kind="ExternalInput").ap()
    wq_d = nc.dram_tensor("wqT", [C, 512], BF16, kind="ExternalInput").ap()
    wkv_d = nc.dram_tensor("wkvT", [C, 256], BF16, kind="ExternalInput").ap()
    wo_d = nc.dram_tensor("woT", [512, C], BF16, kind="ExternalInput").ap()
    cosq_d = nc.dram_tensor("cosqT", [128, T], BF16, kind="ExternalInput").ap()
    sinq_d = nc.dram_tensor("sinqT", [128, T], BF16, kind="ExternalInput").ap()
    cosk_d = nc.dram_tensor("coskT", [128, T], BF16, kind="ExternalInput").ap()
    sink_d = nc.dram_tensor("sinkT", [128, T], BF16, kind="ExternalInput").ap()
    btab_d = nc.dram_tensor("btab", [128, 128], F32, kind="ExternalInput").ap()
    out_d = nc.dram_tensor("out", [T, C], BF16, kind="ExternalOutput").ap()

    with tile.TileContext(nc) as tc:
        cp_cm = tc.tile_pool(name="const", bufs=1)
        cp = cp_cm.__enter__()
        ident = cp.tile([128, 128], BF16, tag="ident")
        causal01 = cp.tile([128, 128], BF16, tag="causal01")
        btab = cp.tile([128, 128], F32, tag="btab")
        # qZ[h]: head h's rope'd q at rows j*64:(j+1)*64 (j = h//4, matching
        # its kv head's rows in kTp), other 64 rows ZERO. Scores then run
        # K=128 full-array mode with kTp as shared stationary: no PE
        # row-tiling mode switches, no swapped-kv copy needed.
        qZ = [cp.tile([128, T], BF16, tag=f"qZ{h}", name=f"qZ{h}")
              for h in range(QH)]
        kTp = cp.tile([128, T], BF16, tag="kTp")
        # v_aug [128k, 4, 64] = [ones | v_kv0 | ones | v_kv1]: each head's
        # stationary slice [ones | v] is contiguous (walrus requires 1 free
        # dim on weights APs); the ones columns make PV replicate the softmax
        # denominator l on psum partitions 0:64 (offset 0 is required by
        # reciprocal_approx_fast, which breaks at offset!=0)
        vA = [cp.tile([128, 4, 64], BF16, tag=f"v{tb}", name=f"v{tb}")
              for tb in range(NTB)]
        # y in hd-major [hd, t] per head-pair, written by normalize()
        yT = [cp.tile([128, T], BF16, tag=f"yT{p}", name=f"yTt{p}")
              for p in range(4)]

        # --------- phase-1 pools (th0 x + th0 tables / weights / tmps) ----
        p1w_cm = tc.tile_pool(name="p1w", bufs=1, side="right")
        p1w = p1w_cm.__enter__()
        p1t_cm = tc.tile_pool(name="p1t", bufs=2, side="right")
        p1t = p1t_cm.__enter__()
        p1x_cm = tc.tile_pool(name="p1x", bufs=1, side="right")
        p1x = p1x_cm.__enter__()
        p1pv_cm = tc.tile_pool(name="p1pv", bufs=1, space="PSUM")
        p1pv = p1pv_cm.__enter__()
        p1ps_cm = tc.tile_pool(name="p1ps", bufs=1, space="PSUM")
        p1ps = p1ps_cm.__enter__()

        # full x rows [128, 1024] (both t-halves at once): 2KB contiguous
        # descriptors instead of 1KB, and the s=0 fillers' xb half is
        # resident long before it's needed
        xf, wqs, wkvs = [], [], []
        for i in range(16):
            xt = p1x.tile([128, 1024], BF16, tag=f"x{i}", name=f"x{i}")
            nc.sync.dma_start(xt[:], xT_d[i * 128:(i + 1) * 128, :])
            xf.append(xt)
            wt = p1w.tile([128, 512], BF16, tag=f"wq{i}", name=f"wq{i}")
            nc.gpsimd.dma_start(wt[:], wq_d[i * 128:(i + 1) * 128, :])
            wqs.append(wt)
            kt = p1w.tile([128, 256], BF16, tag=f"wkv{i}", name=f"wkv{i}")
            nc.scalar.dma_start(kt[:], wkv_d[i * 128:(i + 1) * 128, :])
            wkvs.append(kt)
        tabs = {}
        for nm, dr in (("cq", cosq_d), ("sq", sinq_d), ("ck", cosk_d), ("sk", sink_d)):
            ta = p1x.tile([128, 1024], BF16, tag=f"{nm}t", name=f"{nm}t")
            nc.scalar.dma_start(ta[:], dr[:, :])
            tabs[nm] = ta
        # const-tile init AFTER the hot input DMAs are on the queues: none of
        # these are needed until v_transposes / attention start
        make_identity(nc, ident[:])
        nc.gpsimd.memset(causal01[:], 1.0)
        nc.gpsimd.affine_select(
            out=causal01[:], in_=causal01[:],
            compare_op=mybir.AluOpType.is_ge, fill=0.0, base=0,
            # keep (1.0) where q - k >= 0, else 0  (k = partition, q = free)
            pattern=[[1, 128]], channel_multiplier=-1)
        nc.scalar.dma_start(btab[:], btab_d)
        for tb in range(NTB):
            nc.gpsimd.memset(vA[tb][:, 0, :], 1.0)
            nc.gpsimd.memset(vA[tb][:, 2, :], 1.0)
        for h in range(QH):
            z0 = 64 if h < 4 else 0
            nc.gpsimd.memset(qZ[h][z0:z0 + 64, :], 0.0)

        def rope_blk(pp, blk, th):
            """pp: [128,512] psum with d-major proj; writes qZ/kTp th-slice."""
            tsl = slice(th * 512, (th + 1) * 512)
            cosT = (tabs["cq"] if blk < 4 else tabs["ck"])[:, tsl]
            sinT = (tabs["sq"] if blk < 4 else tabs["sk"])[:, tsl]
            ev = p1t.tile([128, 512], BF16, tag="ev", name="ev")
            nc.scalar.copy(ev[:], pp[:])
            rot = p1t.tile([128, 512], BF16, tag="rot", name="rot")
            for q0 in (0, 64):
                nc.gpsimd.dma_start(rot[q0:q0 + 32, :], ev[q0 + 32:q0 + 64, :])
                nc.gpsimd.dma_start(rot[q0 + 32:q0 + 64, :], ev[q0:q0 + 32, :])
            t1 = p1t.tile([128, 512], BF16, tag="t1", name="t1")
            nc.vector.tensor_mul(t1[:], ev[:], cosT)
            t2 = p1t.tile([128, 512], BF16, tag="t2", name="t2")
            nc.vector.tensor_mul(t2[:], rot[:], sinT)
            if blk < 4:
                rsl = slice((blk // 2) * 64, (blk // 2) * 64 + 64)
                nc.vector.tensor_add(qZ[2 * blk][rsl, tsl],
                                     t1[0:64, :], t2[0:64, :])
                nc.vector.tensor_add(qZ[2 * blk + 1][rsl, tsl],
                                     t1[64:128, :], t2[64:128, :])
            else:
                nc.vector.tensor_add(kTp[:, tsl], t1[:], t2[:])

        def v_evac(pp):
            vsb = p1t.tile([128, 512], BF16, tag="vsb", name="vsb")
            nc.scalar.copy(vsb[:], pp[:])
            return vsb

        def v_transpose_one(th, vsb, qb, pool):
            tb = th * 4 + qb
            vt = pool.tile([128, 128], BF16, tag=pool._vt_tag, name="vt")
            nc.tensor.transpose(vt[:], vsb[:, qb * 128:(qb + 1) * 128],
                                ident[:])
            nc.scalar.copy(vA[tb][:, 1:4:2, :],
                           vt[:].rearrange("p (a b) -> p a b", a=2))

        def blk_w(ci, blk):
            if blk < 4:
                return wqs[ci], slice(blk * 128, (blk + 1) * 128)
            return wkvs[ci], slice((blk - 4) * 128, (blk - 3) * 128)

        # --------- th0 projection: blk-outer so each psum group finishes
        # early and its rope (or v evac) starts while the next group's MMs
        # run; k/v first since they only need wkv+xa
        pps = [p1ps.tile([128, 512], F32, tag=f"pp{b}", name=f"pp{b}")
               for b in range(6)]
        p1pv._vt_tag = "vt"
        # k and v proj interleaved per x tile: the head of phase-1 is paced
        # by x DMA arrival (~550ns/tile vs 216ns/MM), so issue both blocks'
        # MMs per tile to halve the idle while waiting for the next tile
        for ci in range(16):
            for blk in (4, 5):
                w, cols = blk_w(ci, blk)
                nc.tensor.matmul(pps[blk][:], w[:, cols],
                                 xf[ci][:, 0:512],
                                 start=(ci == 0), stop=(ci == 15))
        rope_blk(pps[4], 4, 0)
        vsb0 = v_evac(pps[5])
        for blk in (0, 1, 2, 3):
            for ci in range(16):
                w, cols = blk_w(ci, blk)
                nc.tensor.matmul(pps[blk][:], w[:, cols],
                                 xf[ci][:, 0:512],
                                 start=(ci == 0), stop=(ci == 15))
            rope_blk(pps[blk], blk, 0)
            # one v transpose per q block (strictly BETWEEN accumulation
            # groups): its psum tile (1 buf) gets a full MM group to cover
            # the evac copy latency
            v_transpose_one(0, vsb0, blk, p1pv)

        p1ps_cm.__exit__(None, None, None)

        # --------- attention pools (+ th1 x / tables, DMA'd now) ----------
        ptp_cm = tc.tile_pool(name="ptp", bufs=12)
        ptp = ptp_cm.__enter__()
        p2t_cm = tc.tile_pool(name="p2t", bufs=2)
        p2t = p2t_cm.__enter__()
        psA_cm = tc.tile_pool(name="psA", bufs=4, space="PSUM")
        psA = psA_cm.__enter__()
        psB_cm = tc.tile_pool(name="psB", bufs=3, space="PSUM")
        psB = psB_cm.__enter__()


        def scores(s, h, kc, pts):
            ql = max(0, kc * 128 - s * 512)
            sp = psA.tile([128, 512], F32, tag="sp", name="sp")
            nc.tensor.matmul(
                sp[:, ql:512],
                kTp[:, kc * 128:(kc + 1) * 128],
                qZ[h][:, s * 512 + ql:(s + 1) * 512],
                start=True, stop=True)
            pt = ptp.tile([128, 512], BF16, tag="pt", name="pt")
            c = ql  # multiple of 128
            while c < 512:
                vq = qtile_vq[s * 4 + c // 128]
                ce = c
                while ce < 512 and qtile_vq[s * 4 + ce // 128] == vq:
                    ce += 128
                col = h * 16 + vq * 8 + kc
                nc.scalar.activation(pt[:, c:ce], sp[:, c:ce], AF.Exp,
                                     bias=btab[:, col:col + 1], scale=1.0)
                c = ce
            if s * 4 <= kc < s * 4 + 4:
                # diagonal block: zero the strict upper triangle post-exp
                nc.vector.tensor_mul(pt[:, ql:ql + 128], pt[:, ql:ql + 128],
                                     causal01[:])
            pts[kc] = pt

        def pv(s, h, kc, kcmax, yp, pts):
            j = h // 4
            ql = max(0, kc * 128 - s * 512)
            lhsT = vA[kc][:, 0:2, :] if j == 0 else vA[kc][:, 2:4, :]
            nc.tensor.matmul(
                yp[:, ql:512], lhsT, pts[kc][:, ql:512],
                start=(kc == 0), stop=(kc == kcmax - 1),
                skip_group_check=True)
            pts[kc] = None

        def normalize(s, h, yp):
            # yp [128, 512]: rows 0:64 = softmax denominator l (64 copies),
            # rows 64:128 = unnormalized y (hd-major).
            p, r = h // 2, (h % 2) * 64
            # approx 1/l (~51 ULP), pipelined in two 256-col chunks to halve
            # the critical latency before the yp psum bank can be recycled.
            rls = p2t.tile([128, 512], F32, tag="rls", name="rls")
            for c0 in (0, 256):
                csl = slice(c0, c0 + 256)
                osl = slice(s * 512 + c0, s * 512 + c0 + 256)
                nc.vector.reciprocal_approx_fast(rls[0:64, csl],
                                                 yp[0:64, csl])
                nc.vector.tensor_mul(yT[p][r:r + 64, osl],
                                     yp[64:128, csl], rls[0:64, csl])

        def attention_half(s, fillers=()):
            # fillers: closures emitting independent PE work, spliced between
            # attention matmul groups so the PE never drains on softmax /
            # psum-recycle latency (keeps the HAM clock gate warm too).
            kcmax = 4 * (s + 1)
            fillers = list(fillers)
            fi = 0
            for hp in range(4):  # head pairs, 3-deep lookahead
                h0, h1 = 2 * hp, 2 * hp + 1
                yp0 = psB.tile([128, 512], F32, tag="yp", name="yp0")
                yp1 = psB.tile([128, 512], F32, tag="yp", name="yp1")
                pts0, pts1 = {}, {}
                for k in range(min(3, kcmax)):
                    scores(s, h0, k, pts0)
                    scores(s, h1, k, pts1)
                # spread filler work evenly across pair starts: the pair
                # boundary is where the PE stalls on yp recycle + softmax
                quota = -(-(len(fillers) - fi) // (4 - hp))  # ceil split
                for _ in range(quota):
                    fillers[fi]()
                    fi += 1
                for kc in range(kcmax):
                    if kc + 3 < kcmax:
                        scores(s, h0, kc + 3, pts0)
                        scores(s, h1, kc + 3, pts1)
                    pv(s, h0, kc, kcmax, yp0, pts0)
                    pv(s, h1, kc, kcmax, yp1, pts1)
                normalize(s, h0, yp0)
                normalize(s, h1, yp1)
            for f in fillers[fi:]:
                f()

        def mk_outproj_unit(tb, n):
            def go():
                trow = slice(tb * 128, (tb + 1) * 128)
                op = psA.tile([128, 512], F32, tag="sp", name="op")
                for p in range(4):
                    nc.tensor.matmul(
                        op[:], yT[p][:, trow],
                        wo[p][:, n * 512:(n + 1) * 512],
                        start=(p == 0), stop=(p == 3))
                oe = ost.tile([128, 512], BF16, tag="oe", name="oe")
                nc.vector.tensor_copy(oe[:], op[:])
                (nc.sync if n % 2 == 0 else nc.scalar).dma_start(
                    out_d[trow, n * 512:(n + 1) * 512], oe[:])
            return go

        th1_state = {}

        def mk_proj_blk(blk):
            def go():
                pp = psA.tile([128, 512], F32, tag="sp", name=f"pp1_{blk}")
                for ci in range(16):
                    w, cols = blk_w(ci, blk)
                    nc.tensor.matmul(pp[:], w[:, cols],
                                     xf[ci][:, 512:1024],
                                     start=(ci == 0), stop=(ci == 15))
                if blk == 5:
                    th1_state["vsb"] = v_evac(pp)
                else:
                    rope_blk(pp, blk, 1)
            return go

        def mk_vt(qb):
            def go():
                v_transpose_one(1, th1_state["vsb"], qb, p1pv)
            return go

        # s=0 attention with th1 projection blocks spliced in as PE filler;
        # the 4 v transposes are separate fillers so each one's psum tile
        # (1 buf) gets attention MMs to cover the evac copy latency.
        # Order balances filler TIME per pair-start (quota splits by count:
        # 3,3,2,2): each later pair still gets a ~3.5us proj block, not just
        # ~0.3us transposes — pairs 2-3 otherwise starve the PE on exp lag.
        attention_half(0, [mk_proj_blk(5), mk_proj_blk(4), mk_proj_blk(0),
                           mk_proj_blk(1), mk_vt(0), mk_vt(1),
                           mk_proj_blk(2), mk_vt(2),
                           mk_proj_blk(3), mk_vt(3)])

        p1x_cm.__exit__(None, None, None)
        p1t_cm.__exit__(None, None, None)
        p1w_cm.__exit__(None, None, None)

        p2c_cm = tc.tile_pool(name="p2c", bufs=1, side="right")
        p2c = p2c_cm.__enter__()
        ost_cm = tc.tile_pool(name="ost", bufs=4, side="right")
        ost = ost_cm.__enter__()
        wo = []
        for p in range(4):
            t = p2c.tile([128, C], BF16, tag=f"wo{p}", name=f"wo{p}")
            nc.scalar.dma_start(t[:], wo_d[p * 128:(p + 1) * 128, :])
            wo.append(t)

        # s=1 attention with s=0 out-proj units spliced in as PE filler
        attention_half(1, [mk_outproj_unit(tb, n)
                           for tb in range(4) for n in range(4)])
        for tb in range(4, 8):
            for n in range(4):
                mk_outproj_unit(tb, n)()

        for cm in (ost_cm, p2c_cm, psB_cm, psA_cm,
                   p2t_cm, ptp_cm, p1pv_cm, cp_cm):
            cm.__exit__(None, None, None)

    nc.compile()
    return nc


def make_core_inputs(x, cos, sin, attention_mask, is_vision, Wq, Wk, Wv, Wo,
                     gate, b, g):
    cos_b = np.asarray(cos[b], dtype=np.float32)   # [T, 64]
    sin_b = np.asarray(sin[b], dtype=np.float32)
    sgn = np.concatenate([-np.ones(32), np.ones(32)]).astype(np.float32)
    cosT = np.tile(cos_b.T, (2, 1))                            # [128, T]
    sinT = np.tile(sin_b.T * sgn[:, None], (2, 1))             # [128, T]
    vk = np.asarray(is_vision[b], dtype=np.int32)
    maskneg = np.where(np.asarray(attention_mask[b]) > 0, 0.0, NEG)

    hq0 = QH * g
    btab = np.empty((128, 128), dtype=np.float32)
    for h in range(QH):
        for vq in range(2):
            for kc in range(8):
                col = h * 16 + vq * 8 + kc
                ks = slice(kc * 128, (kc + 1) * 128)
                btab[:, col] = gate[hq0 + h, 2 * vq + vk[ks]] + maskneg[ks]

    return {
        "xT": np.ascontiguousarray(x[b].T).astype(NP_BF16),
        "wqT": np.ascontiguousarray(
            Wq[hq0 * 64:hq0 * 64 + 512, :].T).astype(NP_BF16),
        "wkvT": np.ascontiguousarray(
            np.concatenate([Wk[128 * g:128 * g + 128, :].T,
                            Wv[128 * g:128 * g + 128, :].T],
                           axis=1)).astype(NP_BF16),
        "woT": np.ascontiguousarray(
            Wo[:, hq0 * 64:hq0 * 64 + 512].T).astype(NP_BF16),
        "cosqT": np.ascontiguousarray(cosT * 0.125).astype(NP_BF16),
        "sinqT": np.ascontiguousarray(sinT * 0.125).astype(NP_BF16),
        "coskT": np.ascontiguousarray(cosT).astype(NP_BF16),
        "sinkT": np.ascontiguousarray(sinT).astype(NP_BF16),
        "btab": btab,
    }


def kernel(x, cos, sin, attention_mask, is_vision, Wq, Wk, Wv, Wo, gate):
    x = np.asarray(x, dtype=np.float32)
    cos = np.asarray(cos, dtype=np.float32)
    sin = np.asarray(sin, dtype=np.float32)
    attention_mask = np.asarray(attention_mask, dtype=np.float32)
    is_vision = np.asarray(is_vision)
    Wq = np.asarray(Wq, dtype=np.float32)
    Wk = np.asarray(Wk, dtype=np.float32)
    Wv = np.asarray(Wv, dtype=np.float32)
    Wo = np.asarray(Wo, dtype=np.float32)
    gate = np.asarray(gate, dtype=np.float32)

    # q-side vision flag must be constant within each 128-token tile and
    # identical across batches (holds for the fixed vision-prefix data).
    iv = is_vision.astype(np.int32)
    qtile_vq = []
    for qt in range(NTB):
        blk = iv[:, qt * 128:(qt + 1) * 128]
        assert (blk == blk[0, 0]).all(), "is_vision not 128-tile constant"
        qtile_vq.append(int(blk[0, 0]))

    in_maps = [
        make_core_inputs(x, cos, sin, attention_mask, is_vision,
                         Wq, Wk, Wv, Wo, gate, b=c // 4, g=c % 4)
        for c in range(NCORES)
    ]

    nc = build_program(qtile_vq)
    trace = bool(int(os.environ.get("NANOVLM_TRACE", "0")))
    if trace:
        results = _run_traced(nc, in_maps)
    else:
        results = run_bass_kernel_spmd(nc, in_maps, list(range(NCORES))).results
    out = np.empty((B, T, C), dtype=np.float32)
    for b in range(B):
        out[b] = sum(np.asarray(results[4 * b + g]["out"], dtype=np.float32)
                     for g in range(4))
    return out


def _ensure_ntff_hook():
    """The agent image's antenv lacks axon_hooks; shim it and register the
    ctypes NTFF profile hook against the axon PJRT .so."""
    try:
        from antenv.axon_hooks import get_axon_ntff_profile_hook  # noqa: F401
        return True
    except ImportError:
        pass
    import types

    import antenv

    mod = types.ModuleType("antenv.axon_hooks")
    mod._hook = None

    def set_axon_ntff_profile_hook(h):
        mod._hook = h

    def get_axon_ntff_profile_hook():
        return mod._hook

    mod.set_axon_ntff_profile_hook = set_axon_ntff_profile_hook
    mod.get_axon_ntff_profile_hook = get_axon_ntff_profile_hook
    sys.modules["antenv.axon_hooks"] = mod
    antenv.axon_hooks = mod
    if "/root/.axon_site" not in sys.path:
        sys.path.insert(0, "/root/.axon_site")
    try:
        from trn_agent_boot.trn_boot import _ntff_profile_via_ctypes

        hook = _ntff_profile_via_ctypes("/opt/axon/libaxon_pjrt.so")
    except Exception as e:
        print("ntff hook setup failed:", e)
        return False
    if hook is None:
        return False
    set_axon_ntff_profile_hook(hook)
    return True


def _run_traced(nc, in_maps, trace_core=0):
    import glob
    import tempfile

    from concourse import bass2jax
    from concourse._compat import FishPath
    import gauge.profiler

    if not _ensure_ntff_hook():
        print("no NTFF hook; running untraced")
        return run_bass_kernel_spmd(nc, in_maps, list(range(NCORES))).results

    from antenv.axon_hooks import get_axon_ntff_profile_hook

    hook = get_axon_ntff_profile_hook()
    tmpdir = tempfile.mkdtemp(prefix="nanovlm_prof_")
    with hook(tmpdir, [trace_core]):
        results = bass2jax.run_bass_via_pjrt(nc, in_maps, n_cores=NCORES)
    ntffs = glob.glob(os.path.join(tmpdir, "*_body*.ntff"))
    if not ntffs:
        print("no NTFF produced; files:", os.listdir(tmpdir))
        return results
    profile = gauge.profiler.Profile(
        profile_path=FishPath(tmpdir),
        kernel_dev_mode=True,
        profile_on_exit=False,
        bass_kernel=nc.m,
        offline_processing=True,
        fname="*_body*",
    )
    try:
        pr = profile.to_perfetto(model_index=(trace_core,))
        kernel.last_exec_time_ns = pr[0].exec_time_ns
        kernel.last_trace = pr[0].trace_path
        print(f"HW exec time: {pr[0].exec_time_ns} ns")
        print("trace:", pr[0].trace_path)
    except Exception as e:
        print("perfetto conversion failed:", type(e).__name__, e)
        print("ntff dir:", tmpdir)
    return results



# revision 48
# speedup vs baseline: 1.0502x; 1.0502x over previous
"""NanoVLM GQA attention block on 8 Trainium2 NeuronCores.

Sharding: core c = 4*b + g handles batch b (of 2) and head-group g (of 4):
8 q-heads (global 8g..8g+8) and 2 kv-heads (2g, 2g+1). Each core computes a
partial output restricted to its heads' columns of Wo; the host sums the 4
partials per batch (the tensor-parallel reduce, done host-side).

Device pipeline (per core, bf16 matmuls, fp32 psum accumulation):
  1. proj with WEIGHTS stationary -> q/k/v directly d-major [hd, t] in PSUM,
     blk-outer (k/v first) so each group's rope starts while the next runs
  2. RoPE in d-major: rotate_half = partition 32<->64 block swap done with
     SBUF->SBUF DMAs (sign folded into the bf16 sin table); 1/sqrt(64)
     folded into q's tables; q written into qZ[h] [128, t]: head h's 64
     rows at its kv head's partition range, other 64 rows zero
  3. v transposed back to t-major, one copy into v_aug [128k, 4, 64] =
     [ones | v_kv0 | ones | v_kv1] (each head's stationary slice is
     contiguous; ones put the softmax denominator l on psum rows 0:64,
     offset 0 as required by reciprocal_approx_fast)
  4. scores: sp [128k, 512q] = kTp_chunk^T @ qZ[h] slice at K=128 full-array
     mode (zero q rows kill the other kv head's contribution; no PE
     row-tiling mode switches), causal sub-ranges only; exp on ACT with
     per-partition bias = gate[h, 2vq+vk] + log-mask, writing P^T bf16;
     diagonal block gets a post-exp causal01 multiply
  5. PV: yp [128, 512q] += v_aug^T @ P^T (rows 0:64 = l, 64:128 = y)
  6. normalize: reciprocal_approx_fast(l) -> rls, DVE mul -> yT[hd, t]
  7. out-proj straight from yT: psum [128t, 512n] over 4 head-pair chunks,
     DVE evac bf16, DMA partial out [1024, 2048] on sync/scalar queues
Schedule: th0 proj -> s=0 attention with th1 proj/rope/v spliced as PE
fillers (time-balanced across pair starts) -> s=1 attention with the first
16 out-proj units spliced in -> trailing out-proj units.
"""

import os
import sys

sys.path.insert(0, "/opt/trn_rl_repo")

import numpy as np

import concourse.bacc as bacc
import concourse.mybir as mybir
import concourse.tile as tile
from concourse.bass_utils import run_bass_kernel_spmd
from concourse.masks import make_identity

F32 = mybir.dt.float32
BF16 = mybir.dt.bfloat16
NP_BF16 = mybir.dt.np(mybir.dt.bfloat16)
AF = mybir.ActivationFunctionType
ALU = mybir.AluOpType

B, T, C = 2, 1024, 2048
NH, NKV, HD = 32, 8, 64
QH, KVH = 8, 2          # per-core q-heads / kv-heads
NTB = T // 128          # 8 t-blocks
NCORES = 8
NEG = -1e30


def build_program(qtile_vq):
    """qtile_vq: per 128-token q-tile, the is_vision value (0/1), len 8."""
    nc = bacc.Bacc("TRN2", target_bir_lowering=False, debug=False,
                   num_devices=NCORES)

    xT_d = nc.dram_tensor("xT", [C, T], BF16, kind="ExternalInput").ap()
    wq_d = nc.dram_tensor("wqT", [C, 512], BF16, kind="ExternalInput").ap()
    wkv_d = nc.dram_tensor("wkvT", [C, 256], BF16, kind="ExternalInput").ap()
    wo_d = nc.dram_tensor("woT", [512, C], BF16, kind="ExternalInput").ap()
    cosq_d = nc.dram_tensor("cosqT", [128, T], BF16, kind="ExternalInput").ap()
    sinq_d = nc.dram_tensor("sinqT", [128, T], BF16, kind="ExternalInput").ap()
    cosk_d = nc.dram_tensor("coskT", [128, T], BF16, kind="ExternalInput").ap()
    sink_d = nc.dram_tensor("sinkT", [128, T], BF16, kind="ExternalInput").ap()
    btab_d = nc.dram_tensor("btab", [128, 128], F32, kind="ExternalInput").ap()
    out_d = nc.dram_tensor("out", [T, C], BF16, kind="ExternalOutput").ap()

    with tile.TileContext(nc) as tc:
        cp_cm = tc.tile_pool(name="const", bufs=1)
        cp = cp_cm.__enter__()
        ident = cp.tile([128, 128], BF16, tag="ident")
        causal01 = cp.tile([128, 128], BF16, tag="causal01")
        btab = cp.tile([128, 128], F32, tag="btab")
        # qZ[h]: head h's rope'd q at rows j*64:(j+1)*64 (j = h//4, matching
        # its kv head's rows in kTp), other 64 rows ZERO. Scores then run
        # K=128 full-array mode with kTp as shared stationary: no PE
        # row-tiling mode switches, no swapped-kv copy needed.
        qZ = [cp.tile([128, T], BF16, tag=f"qZ{h}", name=f"qZ{h}")
              for h in range(QH)]
        kTp = cp.tile([128, T], BF16, tag="kTp")
        # v_aug [128k, 4, 64] = [ones | v_kv0 | ones | v_kv1]: each head's
        # stationary slice [ones | v] is contiguous (walrus requires 1 free
        # dim on weights APs); the ones columns make PV replicate the softmax
        # denominator l on psum partitions 0:64 (offset 0 is required by
        # reciprocal_approx_fast, which breaks at offset!=0)
        vA = [cp.tile([128, 4, 64], BF16, tag=f"v{tb}", name=f"v{tb}")
              for tb in range(NTB)]
        # y in hd-major [hd, t] per head-pair, written by normalize()
        yT = [cp.tile([128, T], BF16, tag=f"yT{p}", name=f"yTt{p}")
              for p in range(4)]

        # --------- phase-1 pools (th0 x + th0 tables / weights / tmps) ----
        p1w_cm = tc.tile_pool(name="p1w", bufs=1, side="right")
        p1w = p1w_cm.__enter__()
        p1t_cm = tc.tile_pool(name="p1t", bufs=2, side="right")
        p1t = p1t_cm.__enter__()
        p1x_cm = tc.tile_pool(name="p1x", bufs=1, side="right")
        p1x = p1x_cm.__enter__()
        p1pv_cm = tc.tile_pool(name="p1pv", bufs=1, space="PSUM")
        p1pv = p1pv_cm.__enter__()
        p1ps_cm = tc.tile_pool(name="p1ps", bufs=1, space="PSUM")
        p1ps = p1ps_cm.__enter__()

        # full x rows [128, 1024] (both t-halves at once): 2KB contiguous
        # descriptors instead of 1KB, and the s=0 fillers' xb half is
        # resident long before it's needed
        xf, wqs, wkvs = [], [], []
        for i in range(16):
            xt = p1x.tile([128, 1024], BF16, tag=f"x{i}", name=f"x{i}")
            nc.sync.dma_start(xt[:], xT_d[i * 128:(i + 1) * 128, :])
            xf.append(xt)
            wt = p1w.tile([128, 512], BF16, tag=f"wq{i}", name=f"wq{i}")
            nc.gpsimd.dma_start(wt[:], wq_d[i * 128:(i + 1) * 128, :])
            wqs.append(wt)
            kt = p1w.tile([128, 256], BF16, tag=f"wkv{i}", name=f"wkv{i}")
            nc.scalar.dma_start(kt[:], wkv_d[i * 128:(i + 1) * 128, :])
            wkvs.append(kt)
        tabs = {}
        for nm, dr in (("cq", cosq_d), ("sq", sinq_d), ("ck", cosk_d), ("sk", sink_d)):
            ta = p1x.tile([128, 1024], BF16, tag=f"{nm}t", name=f"{nm}t")
            nc.scalar.dma_start(ta[:], dr[:, :])
            tabs[nm] = ta
        # const-tile init AFTER the hot input DMAs are on the queues: none of
        # these are needed until v_transposes / attention start
        make_identity(nc, ident[:])
        nc.gpsimd.memset(causal01[:], 1.0)
        nc.gpsimd.affine_select(
            out=causal01[:], in_=causal01[:],
            compare_op=mybir.AluOpType.is_ge, fill=0.0, base=0,
            # keep (1.0) where q - k >= 0, else 0  (k = partition, q = free)
            pattern=[[1, 128]], channel_multiplier=-1)
        nc.scalar.dma_start(btab[:], btab_d)
        for tb in range(NTB):
            nc.gpsimd.memset(vA[tb][:, 0, :], 1.0)
            nc.gpsimd.memset(vA[tb][:, 2, :], 1.0)
        for h in range(QH):
            z0 = 64 if h < 4 else 0
            nc.gpsimd.memset(qZ[h][z0:z0 + 64, :], 0.0)

        def rope_blk(pp, blk, th):
            """pp: [128,512] psum with d-major proj; writes qZ/kTp th-slice."""
            tsl = slice(th * 512, (th + 1) * 512)
            cosT = (tabs["cq"] if blk < 4 else tabs["ck"])[:, tsl]
            sinT = (tabs["sq"] if blk < 4 else tabs["sk"])[:, tsl]
            ev = p1t.tile([128, 512], BF16, tag="ev", name="ev")
            nc.scalar.copy(ev[:], pp[:])
            rot = p1t.tile([128, 512], BF16, tag="rot", name="rot")
            for q0 in (0, 64):
                nc.gpsimd.dma_start(rot[q0:q0 + 32, :], ev[q0 + 32:q0 + 64, :])
                nc.gpsimd.dma_start(rot[q0 + 32:q0 + 64, :], ev[q0:q0 + 32, :])
            t1 = p1t.tile([128, 512], BF16, tag="t1", name="t1")
            nc.vector.tensor_mul(t1[:], ev[:], cosT)
            t2 = p1t.tile([128, 512], BF16, tag="t2", name="t2")
            nc.vector.tensor_mul(t2[:], rot[:], sinT)
            if blk < 4:
                rsl = slice((blk // 2) * 64, (blk // 2) * 64 + 64)
                nc.vector.tensor_add(qZ[2 * blk][rsl, tsl],
                                     t1[0:64, :], t2[0:64, :])
                nc.vector.tensor_add(qZ[2 * blk + 1][rsl, tsl],
                                     t1[64:128, :], t2[64:128, :])
            else:
                nc.vector.tensor_add(kTp[:, tsl], t1[:], t2[:])

        def v_evac(pp):
            vsb = p1t.tile([128, 512], BF16, tag="vsb", name="vsb")
            nc.scalar.copy(vsb[:], pp[:])
            return vsb

        def v_transpose_one(th, vsb, qb, pool):
            tb = th * 4 + qb
            vt = pool.tile([128, 128], BF16, tag=pool._vt_tag, name="vt")
            nc.tensor.transpose(vt[:], vsb[:, qb * 128:(qb + 1) * 128],
                                ident[:])
            nc.scalar.copy(vA[tb][:, 1:4:2, :],
                           vt[:].rearrange("p (a b) -> p a b", a=2))

        def blk_w(ci, blk):
            if blk < 4:
                return wqs[ci], slice(blk * 128, (blk + 1) * 128)
            return wkvs[ci], slice((blk - 4) * 128, (blk - 3) * 128)

        # --------- th0 projection: blk-outer so each psum group finishes
        # early and its rope (or v evac) starts while the next group's MMs
        # run; k/v first since they only need wkv+xa
        pps = [p1ps.tile([128, 512], F32, tag=f"pp{b}", name=f"pp{b}")
               for b in range(6)]
        p1pv._vt_tag = "vt"
        # k and v proj interleaved per x tile: the head of phase-1 is paced
        # by x DMA arrival (~550ns/tile vs 216ns/MM), so issue both blocks'
        # MMs per tile to halve the idle while waiting for the next tile
        for ci in range(16):
            for blk in (4, 5):
                w, cols = blk_w(ci, blk)
                nc.tensor.matmul(pps[blk][:], w[:, cols],
                                 xf[ci][:, 0:512],
                                 start=(ci == 0), stop=(ci == 15))
        rope_blk(pps[4], 4, 0)
        vsb0 = v_evac(pps[5])
        for blk in (0, 1, 2, 3):
            for ci in range(16):
                w, cols = blk_w(ci, blk)
                nc.tensor.matmul(pps[blk][:], w[:, cols],
                                 xf[ci][:, 0:512],
                                 start=(ci == 0), stop=(ci == 15))
            rope_blk(pps[blk], blk, 0)
            # one v transpose per q block (strictly BETWEEN accumulation
            # groups): its psum tile (1 buf) gets a full MM group to cover
            # the evac copy latency
            v_transpose_one(0, vsb0, blk, p1pv)

        p1ps_cm.__exit__(None, None, None)

        # --------- attention pools (+ th1 x / tables, DMA'd now) ----------
        ptp_cm = tc.tile_pool(name="ptp", bufs=14)
        ptp = ptp_cm.__enter__()
        p2t_cm = tc.tile_pool(name="p2t", bufs=2)
        p2t = p2t_cm.__enter__()
        psA_cm = tc.tile_pool(name="psA", bufs=4, space="PSUM")
        psA = psA_cm.__enter__()
        psB_cm = tc.tile_pool(name="psB", bufs=3, space="PSUM")
        psB = psB_cm.__enter__()


        def scores(s, h, kc, pts):
            ql = max(0, kc * 128 - s * 512)
            sp = psA.tile([128, 512], F32, tag="sp", name="sp")
            nc.tensor.matmul(
                sp[:, ql:512],
                kTp[:, kc * 128:(kc + 1) * 128],
                qZ[h][:, s * 512 + ql:(s + 1) * 512],
                start=True, stop=True)
            pt = ptp.tile([128, 512], BF16, tag="pt", name="pt")
            c = ql  # multiple of 128
            while c < 512:
                vq = qtile_vq[s * 4 + c // 128]
                ce = c
                while ce < 512 and qtile_vq[s * 4 + ce // 128] == vq:
                    ce += 128
                col = h * 16 + vq * 8 + kc
                nc.scalar.activation(pt[:, c:ce], sp[:, c:ce], AF.Exp,
                                     bias=btab[:, col:col + 1], scale=1.0)
                c = ce
            if s * 4 <= kc < s * 4 + 4:
                # diagonal block: zero the strict upper triangle post-exp
                nc.vector.tensor_mul(pt[:, ql:ql + 128], pt[:, ql:ql + 128],
                                     causal01[:])
            pts[kc] = pt

        def pv(s, h, kc, kcmax, yp, pts):
            j = h // 4
            ql = max(0, kc * 128 - s * 512)
            lhsT = vA[kc][:, 0:2, :] if j == 0 else vA[kc][:, 2:4, :]
            nc.tensor.matmul(
                yp[:, ql:512], lhsT, pts[kc][:, ql:512],
                start=(kc == 0), stop=(kc == kcmax - 1),
                skip_group_check=True)
            pts[kc] = None

        def normalize(s, h, yp):
            # yp [128, 512]: rows 0:64 = softmax denominator l (64 copies),
            # rows 64:128 = unnormalized y (hd-major).
            p, r = h // 2, (h % 2) * 64
            # approx 1/l (~51 ULP), pipelined in two 256-col chunks to halve
            # the critical latency before the yp psum bank can be recycled.
            rls = p2t.tile([128, 512], F32, tag="rls", name="rls")
            for c0 in (0, 256):
                csl = slice(c0, c0 + 256)
                osl = slice(s * 512 + c0, s * 512 + c0 + 256)
                nc.vector.reciprocal_approx_fast(rls[0:64, csl],
                                                 yp[0:64, csl])
                nc.vector.tensor_mul(yT[p][r:r + 64, osl],
                                     yp[64:128, csl], rls[0:64, csl])

        def attention_half(s, fillers=(), pre=None):
            # fillers: closures emitting independent PE work, spliced between
            # attention matmul groups so the PE never drains on softmax /
            # psum-recycle latency (keeps the HAM clock gate warm too).
            # pre: optional (pts0, pts1) with pair 0's kc 0..2 score tiles
            # already emitted (as late fillers of the PREVIOUS half, filling
            # the PE while that half's exp backlog drains).
            kcmax = 4 * (s + 1)
            fillers = list(fillers)
            fi = 0
            for hp in range(4):  # head pairs, 3-deep lookahead
                h0, h1 = 2 * hp, 2 * hp + 1
                yp0 = psB.tile([128, 512], F32, tag="yp", name="yp0")
                yp1 = psB.tile([128, 512], F32, tag="yp", name="yp1")
                if hp == 0 and pre is not None:
                    pts0, pts1 = pre
                else:
                    pts0, pts1 = {}, {}
                    for k in range(min(3, kcmax)):
                        scores(s, h0, k, pts0)
                        scores(s, h1, k, pts1)
                # spread filler work evenly across pair starts: the pair
                # boundary is where the PE stalls on yp recycle + softmax
                quota = -(-(len(fillers) - fi) // (4 - hp))  # ceil split
                for _ in range(quota):
                    fillers[fi]()
                    fi += 1
                for kc in range(kcmax):
                    if kc + 3 < kcmax:
                        scores(s, h0, kc + 3, pts0)
                        scores(s, h1, kc + 3, pts1)
                    pv(s, h0, kc, kcmax, yp0, pts0)
                    pv(s, h1, kc, kcmax, yp1, pts1)
                normalize(s, h0, yp0)
                normalize(s, h1, yp1)
            for f in fillers[fi:]:
                f()

        def mk_outproj_unit(tb, n):
            def go():
                trow = slice(tb * 128, (tb + 1) * 128)
                op = psA.tile([128, 512], F32, tag="sp", name="op")
                for p in range(4):
                    nc.tensor.matmul(
                        op[:], yT[p][:, trow],
                        wo[p][:, n * 512:(n + 1) * 512],
                        start=(p == 0), stop=(p == 3))
                oe = ost.tile([128, 512], BF16, tag="oe", name="oe")
                nc.vector.tensor_copy(oe[:], op[:])
                (nc.sync if n % 2 == 0 else nc.scalar).dma_start(
                    out_d[trow, n * 512:(n + 1) * 512], oe[:])
            return go

        th1_state = {}

        def mk_proj_blk(blk):
            def go():
                pp = psA.tile([128, 512], F32, tag="sp", name=f"pp1_{blk}")
                for ci in range(16):
                    w, cols = blk_w(ci, blk)
                    nc.tensor.matmul(pp[:], w[:, cols],
                                     xf[ci][:, 512:1024],
                                     start=(ci == 0), stop=(ci == 15))
                if blk == 5:
                    th1_state["vsb"] = v_evac(pp)
                else:
                    rope_blk(pp, blk, 1)
            return go

        def mk_vt(qb):
            def go():
                v_transpose_one(1, th1_state["vsb"], qb, p1pv)
            return go

        # s=0 attention with th1 projection blocks spliced in as PE filler;
        # the 4 v transposes are separate fillers so each one's psum tile
        # (1 buf) gets attention MMs to cover the evac copy latency.
        # Order balances filler TIME per pair-start (quota splits by count:
        # 3,3,2,2): each later pair still gets a ~3.5us proj block, not just
        # ~0.3us transposes — pairs 2-3 otherwise starve the PE on exp lag.
        pre1 = ({}, {})

        def mk_s1_scores(h, kc):
            def go():
                scores(1, h, kc, pre1[h])
            return go

        attention_half(0, [mk_proj_blk(5), mk_proj_blk(4), mk_proj_blk(0),
                           mk_proj_blk(1), mk_vt(0), mk_vt(1),
                           mk_proj_blk(2), mk_vt(2),
                           mk_proj_blk(3), mk_vt(3)]
                       + [mk_s1_scores(h, kc)
                          for kc in range(3) for h in (0, 1)])

        p1x_cm.__exit__(None, None, None)
        p1t_cm.__exit__(None, None, None)
        p1w_cm.__exit__(None, None, None)

        p2c_cm = tc.tile_pool(name="p2c", bufs=1, side="right")
        p2c = p2c_cm.__enter__()
        ost_cm = tc.tile_pool(name="ost", bufs=4, side="right")
        ost = ost_cm.__enter__()
        wo = []
        for p in range(4):
            t = p2c.tile([128, C], BF16, tag=f"wo{p}", name=f"wo{p}")
            nc.scalar.dma_start(t[:], wo_d[p * 128:(p + 1) * 128, :])
            wo.append(t)

        # s=1 attention with s=0 out-proj units spliced in as PE filler
        attention_half(1, [mk_outproj_unit(tb, n)
                           for tb in range(4) for n in range(4)],
                       pre=pre1)
        for tb in range(4, 8):
            for n in range(4):
                mk_outproj_unit(tb, n)()

        for cm in (ost_cm, p2c_cm, psB_cm, psA_cm,
                   p2t_cm, ptp_cm, p1pv_cm, cp_cm):
            cm.__exit__(None, None, None)

    nc.compile()
    return nc


def make_core_inputs(x, cos, sin, attention_mask, is_vision, Wq, Wk, Wv, Wo,
                     gate, b, g):
    cos_b = np.asarray(cos[b], dtype=np.float32)   # [T, 64]
    sin_b = np.asarray(sin[b], dtype=np.float32)
    sgn = np.concatenate([-np.ones(32), np.ones(32)]).astype(np.float32)
    cosT = np.tile(cos_b.T, (2, 1))                            # [128, T]
    sinT = np.tile(sin_b.T * sgn[:, None], (2, 1))             # [128, T]
    vk = np.asarray(is_vision[b], dtype=np.int32)
    maskneg = np.where(np.asarray(attention_mask[b]) > 0, 0.0, NEG)

    hq0 = QH * g
    btab = np.empty((128, 128), dtype=np.float32)
    for h in range(QH):
        for vq in range(2):
            for kc in range(8):
                col = h * 16 + vq * 8 + kc
                ks = slice(kc * 128, (kc + 1) * 128)
                btab[:, col] = gate[hq0 + h, 2 * vq + vk[ks]] + maskneg[ks]

    return {
        "xT": np.ascontiguousarray(x[b].T).astype(NP_BF16),
        "wqT": np.ascontiguousarray(
            Wq[hq0 * 64:hq0 * 64 + 512, :].T).astype(NP_BF16),
        "wkvT": np.ascontiguousarray(
            np.concatenate([Wk[128 * g:128 * g + 128, :].T,
                            Wv[128 * g:128 * g + 128, :].T],
                           axis=1)).astype(NP_BF16),
        "woT": np.ascontiguousarray(
            Wo[:, hq0 * 64:hq0 * 64 + 512].T).astype(NP_BF16),
        "cosqT": np.ascontiguousarray(cosT * 0.125).astype(NP_BF16),
        "sinqT": np.ascontiguousarray(sinT * 0.125).astype(NP_BF16),
        "coskT": np.ascontiguousarray(cosT).astype(NP_BF16),
        "sinkT": np.ascontiguousarray(sinT).astype(NP_BF16),
        "btab": btab,
    }


def kernel(x, cos, sin, attention_mask, is_vision, Wq, Wk, Wv, Wo, gate):
    x = np.asarray(x, dtype=np.float32)
    cos = np.asarray(cos, dtype=np.float32)
    sin = np.asarray(sin, dtype=np.float32)
    attention_mask = np.asarray(attention_mask, dtype=np.float32)
    is_vision = np.asarray(is_vision)
    Wq = np.asarray(Wq, dtype=np.float32)
    Wk = np.asarray(Wk, dtype=np.float32)
    Wv = np.asarray(Wv, dtype=np.float32)
    Wo = np.asarray(Wo, dtype=np.float32)
    gate = np.asarray(gate, dtype=np.float32)

    # q-side vision flag must be constant within each 128-token tile and
    # identical across batches (holds for the fixed vision-prefix data).
    iv = is_vision.astype(np.int32)
    qtile_vq = []
    for qt in range(NTB):
        blk = iv[:, qt * 128:(qt + 1) * 128]
        assert (blk == blk[0, 0]).all(), "is_vision not 128-tile constant"
        qtile_vq.append(int(blk[0, 0]))

    in_maps = [
        make_core_inputs(x, cos, sin, attention_mask, is_vision,
                         Wq, Wk, Wv, Wo, gate, b=c // 4, g=c % 4)
        for c in range(NCORES)
    ]

    nc = build_program(qtile_vq)
    trace = bool(int(os.environ.get("NANOVLM_TRACE", "0")))
    if trace:
        results = _run_traced(nc, in_maps)
    else:
        results = run_bass_kernel_spmd(nc, in_maps, list(range(NCORES))).results
    out = np.empty((B, T, C), dtype=np.float32)
    for b in range(B):
        out[b] = sum(np.asarray(results[4 * b + g]["out"], dtype=np.float32)
                     for g in range(4))
    return out


def _ensure_ntff_hook():
    """The agent image's antenv lacks axon_hooks; shim it and register the
    ctypes NTFF profile hook against the axon PJRT .so."""
    try:
        from antenv.axon_hooks import get_axon_ntff_profile_hook  # noqa: F401
        return True
    except ImportError:
        pass
    import types

    import antenv

    mod = types.ModuleType("antenv.axon_hooks")
    mod._hook = None

    def set_axon_ntff_profile_hook(h):
        mod._hook = h

    def get_axon_ntff_profile_hook():
        return mod._hook

    mod.set_axon_ntff_profile_hook = set_axon_ntff_profile_hook
    mod.get_axon_ntff_profile_hook = get_axon_ntff_profile_hook
    sys.modules["antenv.axon_hooks"] = mod
    antenv.axon_hooks = mod
    if "/root/.axon_site" not in sys.path:
        sys.path.insert(0, "/root/.axon_site")
    try:
        from trn_agent_boot.trn_boot import _ntff_profile_via_ctypes

        hook = _ntff_profile_via_ctypes("/opt/axon/libaxon_pjrt.so")
    except Exception as e:
        print("ntff hook setup failed:", e)
        return False
    if hook is None:
        return False
    set_axon_ntff_profile_hook(hook)
    return True


def _run_traced(nc, in_maps, trace_core=0):
    import glob
    import tempfile

    from concourse import bass2jax
    from concourse._compat import FishPath
    import gauge.profiler

    if not _ensure_ntff_hook():
        print("no NTFF hook; running untraced")
        return run_bass_kernel_spmd(nc, in_maps, list(range(NCORES))).results

    from antenv.axon_hooks import get_axon_ntff_profile_hook

    hook = get_axon_ntff_profile_hook()
    tmpdir = tempfile.mkdtemp(prefix="nanovlm_prof_")
    with hook(tmpdir, [trace_core]):
        results = bass2jax.run_bass_via_pjrt(nc, in_maps, n_cores=NCORES)
    ntffs = glob.glob(os.path.join(tmpdir, "*_body*.ntff"))
    if not ntffs:
        print("no NTFF produced; files:", os.listdir(tmpdir))
        return results
    profile = gauge.profiler.Profile(
        profile_path=FishPath(tmpdir),
        kernel_dev_mode=True,
        profile_on_exit=False,
        bass_kernel=nc.m,
        offline_processing=True,
        fname="*_body*",
    )
    try:
        pr = profile.to_perfetto(model_index=(trace_core,))
        kernel.last_exec_time_ns = pr[0].exec_time_ns
        kernel.last_trace = pr[0].trace_path
        print(f"HW exec time: {pr[0].exec_time_ns} ns")
        print("trace:", pr[0].trace_path)
    except Exception as e:
        print("perfetto conversion failed:", type(e).__name__, e)
        print("ntff dir:", tmpdir)
    return results



# revision 50
# speedup vs baseline: 1.0569x; 1.0064x over previous
"""NanoVLM GQA attention block on 8 Trainium2 NeuronCores.

Sharding: core c = 4*b + g handles batch b (of 2) and head-group g (of 4):
8 q-heads (global 8g..8g+8) and 2 kv-heads (2g, 2g+1). Each core computes a
partial output restricted to its heads' columns of Wo; the host sums the 4
partials per batch (the tensor-parallel reduce, done host-side).

Device pipeline (per core, bf16 matmuls, fp32 psum accumulation):
  1. proj with WEIGHTS stationary -> q/k/v directly d-major [hd, t] in PSUM,
     blk-outer (k/v first) so each group's rope starts while the next runs
  2. RoPE in d-major: rotate_half = partition 32<->64 block swap done with
     SBUF->SBUF DMAs (sign folded into the bf16 sin table); 1/sqrt(64)
     folded into q's tables; q written into qZ[h] [128, t]: head h's 64
     rows at its kv head's partition range, other 64 rows zero
  3. v transposed back to t-major, one copy into v_aug [128k, 4, 64] =
     [ones | v_kv0 | ones | v_kv1] (each head's stationary slice is
     contiguous; ones put the softmax denominator l on psum rows 0:64,
     offset 0 as required by reciprocal_approx_fast)
  4. scores: sp [128k, 512q] = kTp_chunk^T @ qZ[h] slice at K=128 full-array
     mode (zero q rows kill the other kv head's contribution; no PE
     row-tiling mode switches), causal sub-ranges only; exp on ACT with
     per-partition bias = gate[h, 2vq+vk] + log-mask, writing P^T bf16;
     diagonal block gets a post-exp causal01 multiply
  5. PV: yp [128, 512q] += v_aug^T @ P^T (rows 0:64 = l, 64:128 = y)
  6. normalize: reciprocal_approx_fast(l) -> rls, DVE mul -> yT[hd, t]
  7. out-proj straight from yT: psum [128t, 512n] over 4 head-pair chunks,
     DVE evac bf16, DMA partial out [1024, 2048] on sync/scalar queues
Schedule: th0 proj -> s=0 attention with th1 proj/rope/v spliced as PE
fillers (time-balanced across pair starts) -> s=1 attention with the first
16 out-proj units spliced in -> trailing out-proj units.
"""

import os
import sys

sys.path.insert(0, "/opt/trn_rl_repo")

import numpy as np

import concourse.bacc as bacc
import concourse.mybir as mybir
import concourse.tile as tile
from concourse.bass_utils import run_bass_kernel_spmd
from concourse.masks import make_identity

F32 = mybir.dt.float32
BF16 = mybir.dt.bfloat16
NP_BF16 = mybir.dt.np(mybir.dt.bfloat16)
AF = mybir.ActivationFunctionType
ALU = mybir.AluOpType

B, T, C = 2, 1024, 2048
NH, NKV, HD = 32, 8, 64
QH, KVH = 8, 2          # per-core q-heads / kv-heads
NTB = T // 128          # 8 t-blocks
NCORES = 8
NEG = -1e30


def build_program(qtile_vq):
    """qtile_vq: per 128-token q-tile, the is_vision value (0/1), len 8."""
    nc = bacc.Bacc("TRN2", target_bir_lowering=False, debug=False,
                   num_devices=NCORES)

    xT_d = nc.dram_tensor("xT", [C, T], BF16, kind="ExternalInput").ap()
    wq_d = nc.dram_tensor("wqT", [C, 512], BF16, kind="ExternalInput").ap()
    wkv_d = nc.dram_tensor("wkvT", [C, 256], BF16, kind="ExternalInput").ap()
    wo_d = nc.dram_tensor("woT", [512, C], BF16, kind="ExternalInput").ap()
    cosq_d = nc.dram_tensor("cosqT", [128, T], BF16, kind="ExternalInput").ap()
    sinq_d = nc.dram_tensor("sinqT", [128, T], BF16, kind="ExternalInput").ap()
    cosk_d = nc.dram_tensor("coskT", [128, T], BF16, kind="ExternalInput").ap()
    sink_d = nc.dram_tensor("sinkT", [128, T], BF16, kind="ExternalInput").ap()
    btab_d = nc.dram_tensor("btab", [128, 128], F32, kind="ExternalInput").ap()
    out_d = nc.dram_tensor("out", [T, C], BF16, kind="ExternalOutput").ap()

    with tile.TileContext(nc) as tc:
        cp_cm = tc.tile_pool(name="const", bufs=1)
        cp = cp_cm.__enter__()
        ident = cp.tile([128, 128], BF16, tag="ident")
        causal01 = cp.tile([128, 128], BF16, tag="causal01")
        btab = cp.tile([128, 128], F32, tag="btab")
        # qZ[h]: head h's rope'd q at rows j*64:(j+1)*64 (j = h//4, matching
        # its kv head's rows in kTp), other 64 rows ZERO. Scores then run
        # K=128 full-array mode with kTp as shared stationary: no PE
        # row-tiling mode switches, no swapped-kv copy needed.
        qZ = [cp.tile([128, T], BF16, tag=f"qZ{h}", name=f"qZ{h}")
              for h in range(QH)]
        kTp = cp.tile([128, T], BF16, tag="kTp")
        # v_aug [128k, 4, 64] = [ones | v_kv0 | ones | v_kv1]: each head's
        # stationary slice [ones | v] is contiguous (walrus requires 1 free
        # dim on weights APs); the ones columns make PV replicate the softmax
        # denominator l on psum partitions 0:64 (offset 0 is required by
        # reciprocal_approx_fast, which breaks at offset!=0)
        vA = [cp.tile([128, 4, 64], BF16, tag=f"v{tb}", name=f"v{tb}")
              for tb in range(NTB)]
        # y in hd-major [hd, t] per head-pair, written by normalize()
        yT = [cp.tile([128, T], BF16, tag=f"yT{p}", name=f"yTt{p}")
              for p in range(4)]

        # --------- phase-1 pools (th0 x + th0 tables / weights / tmps) ----
        p1w_cm = tc.tile_pool(name="p1w", bufs=1, side="right")
        p1w = p1w_cm.__enter__()
        p1t_cm = tc.tile_pool(name="p1t", bufs=2, side="right")
        p1t = p1t_cm.__enter__()
        p1x_cm = tc.tile_pool(name="p1x", bufs=1, side="right")
        p1x = p1x_cm.__enter__()
        p1pv_cm = tc.tile_pool(name="p1pv", bufs=1, space="PSUM")
        p1pv = p1pv_cm.__enter__()
        p1ps_cm = tc.tile_pool(name="p1ps", bufs=1, space="PSUM")
        p1ps = p1ps_cm.__enter__()

        # full x rows [128, 1024] (both t-halves at once): 2KB contiguous
        # descriptors instead of 1KB, and the s=0 fillers' xb half is
        # resident long before it's needed
        xf, wqs, wkvs = [], [], []
        for i in range(16):
            xt = p1x.tile([128, 1024], BF16, tag=f"x{i}", name=f"x{i}")
            nc.sync.dma_start(xt[:], xT_d[i * 128:(i + 1) * 128, :])
            xf.append(xt)
            wt = p1w.tile([128, 512], BF16, tag=f"wq{i}", name=f"wq{i}")
            nc.gpsimd.dma_start(wt[:], wq_d[i * 128:(i + 1) * 128, :])
            wqs.append(wt)
            kt = p1w.tile([128, 256], BF16, tag=f"wkv{i}", name=f"wkv{i}")
            nc.scalar.dma_start(kt[:], wkv_d[i * 128:(i + 1) * 128, :])
            wkvs.append(kt)
        tabs = {}
        for nm, dr in (("cq", cosq_d), ("sq", sinq_d), ("ck", cosk_d), ("sk", sink_d)):
            ta = p1x.tile([128, 1024], BF16, tag=f"{nm}t", name=f"{nm}t")
            nc.scalar.dma_start(ta[:], dr[:, :])
            tabs[nm] = ta
        # const-tile init AFTER the hot input DMAs are on the queues: none of
        # these are needed until v_transposes / attention start
        make_identity(nc, ident[:])
        nc.gpsimd.memset(causal01[:], 1.0)
        nc.gpsimd.affine_select(
            out=causal01[:], in_=causal01[:],
            compare_op=mybir.AluOpType.is_ge, fill=0.0, base=0,
            # keep (1.0) where q - k >= 0, else 0  (k = partition, q = free)
            pattern=[[1, 128]], channel_multiplier=-1)
        nc.scalar.dma_start(btab[:], btab_d)
        for tb in range(NTB):
            nc.gpsimd.memset(vA[tb][:, 0, :], 1.0)
            nc.gpsimd.memset(vA[tb][:, 2, :], 1.0)
        for h in range(QH):
            z0 = 64 if h < 4 else 0
            nc.gpsimd.memset(qZ[h][z0:z0 + 64, :], 0.0)
        # tiny dummy exp: hoists the 1.28us ACT_TABLE_LOAD into the initial
        # DMA-wait dead time, off the first real exp -> first PV path
        zz = cp.tile([1, 2], F32, tag="zz", name="zz")
        nc.gpsimd.memset(zz[:], 0.0)
        nc.scalar.activation(zz[0:1, 1:2], zz[0:1, 0:1], AF.Exp,
                             bias=zz[0:1, 0:1], scale=1.0)

        def rope_blk(pp, blk, th):
            """pp: [128,512] psum with d-major proj; writes qZ/kTp th-slice."""
            tsl = slice(th * 512, (th + 1) * 512)
            cosT = (tabs["cq"] if blk < 4 else tabs["ck"])[:, tsl]
            sinT = (tabs["sq"] if blk < 4 else tabs["sk"])[:, tsl]
            ev = p1t.tile([128, 512], BF16, tag="ev", name="ev")
            nc.scalar.copy(ev[:], pp[:])
            rot = p1t.tile([128, 512], BF16, tag="rot", name="rot")
            for q0 in (0, 64):
                nc.gpsimd.dma_start(rot[q0:q0 + 32, :], ev[q0 + 32:q0 + 64, :])
                nc.gpsimd.dma_start(rot[q0 + 32:q0 + 64, :], ev[q0:q0 + 32, :])
            t1 = p1t.tile([128, 512], BF16, tag="t1", name="t1")
            nc.vector.tensor_mul(t1[:], ev[:], cosT)
            t2 = p1t.tile([128, 512], BF16, tag="t2", name="t2")
            nc.vector.tensor_mul(t2[:], rot[:], sinT)
            if blk < 4:
                rsl = slice((blk // 2) * 64, (blk // 2) * 64 + 64)
                nc.vector.tensor_add(qZ[2 * blk][rsl, tsl],
                                     t1[0:64, :], t2[0:64, :])
                nc.vector.tensor_add(qZ[2 * blk + 1][rsl, tsl],
                                     t1[64:128, :], t2[64:128, :])
            else:
                nc.vector.tensor_add(kTp[:, tsl], t1[:], t2[:])

        def v_evac(pp):
            vsb = p1t.tile([128, 512], BF16, tag="vsb", name="vsb")
            nc.scalar.copy(vsb[:], pp[:])
            return vsb

        def v_transpose_one(th, vsb, qb, pool):
            tb = th * 4 + qb
            vt = pool.tile([128, 128], BF16, tag=pool._vt_tag, name="vt")
            nc.tensor.transpose(vt[:], vsb[:, qb * 128:(qb + 1) * 128],
                                ident[:])
            nc.scalar.copy(vA[tb][:, 1:4:2, :],
                           vt[:].rearrange("p (a b) -> p a b", a=2))

        def blk_w(ci, blk):
            if blk < 4:
                return wqs[ci], slice(blk * 128, (blk + 1) * 128)
            return wkvs[ci], slice((blk - 4) * 128, (blk - 3) * 128)

        # --------- th0 projection: blk-outer so each psum group finishes
        # early and its rope (or v evac) starts while the next group's MMs
        # run; k/v first since they only need wkv+xa
        pps = [p1ps.tile([128, 512], F32, tag=f"pp{b}", name=f"pp{b}")
               for b in range(6)]
        p1pv._vt_tag = "vt"
        # k and v proj interleaved per x tile: the head of phase-1 is paced
        # by x DMA arrival (~550ns/tile vs 216ns/MM), so issue both blocks'
        # MMs per tile to halve the idle while waiting for the next tile
        for ci in range(16):
            for blk in (4, 5):
                w, cols = blk_w(ci, blk)
                nc.tensor.matmul(pps[blk][:], w[:, cols],
                                 xf[ci][:, 0:512],
                                 start=(ci == 0), stop=(ci == 15))
        rope_blk(pps[4], 4, 0)
        vsb0 = v_evac(pps[5])
        for blk in (0, 1, 2, 3):
            for ci in range(16):
                w, cols = blk_w(ci, blk)
                nc.tensor.matmul(pps[blk][:], w[:, cols],
                                 xf[ci][:, 0:512],
                                 start=(ci == 0), stop=(ci == 15))
            rope_blk(pps[blk], blk, 0)
            # one v transpose per q block (strictly BETWEEN accumulation
            # groups): its psum tile (1 buf) gets a full MM group to cover
            # the evac copy latency
            v_transpose_one(0, vsb0, blk, p1pv)

        p1ps_cm.__exit__(None, None, None)

        # --------- attention pools (+ th1 x / tables, DMA'd now) ----------
        ptp_cm = tc.tile_pool(name="ptp", bufs=12)
        ptp = ptp_cm.__enter__()
        p2t_cm = tc.tile_pool(name="p2t", bufs=2)
        p2t = p2t_cm.__enter__()
        psA_cm = tc.tile_pool(name="psA", bufs=4, space="PSUM")
        psA = psA_cm.__enter__()
        psB_cm = tc.tile_pool(name="psB", bufs=3, space="PSUM")
        psB = psB_cm.__enter__()


        def scores(s, h, kc, pts):
            ql = max(0, kc * 128 - s * 512)
            sp = psA.tile([128, 512], F32, tag="sp", name="sp")
            nc.tensor.matmul(
                sp[:, ql:512],
                kTp[:, kc * 128:(kc + 1) * 128],
                qZ[h][:, s * 512 + ql:(s + 1) * 512],
                start=True, stop=True)
            pt = ptp.tile([128, 512], BF16, tag="pt", name="pt")
            c = ql  # multiple of 128
            while c < 512:
                vq = qtile_vq[s * 4 + c // 128]
                ce = c
                while ce < 512 and qtile_vq[s * 4 + ce // 128] == vq:
                    ce += 128
                col = h * 16 + vq * 8 + kc
                nc.scalar.activation(pt[:, c:ce], sp[:, c:ce], AF.Exp,
                                     bias=btab[:, col:col + 1], scale=1.0)
                c = ce
            if s * 4 <= kc < s * 4 + 4:
                # diagonal block: zero the strict upper triangle post-exp
                nc.vector.tensor_mul(pt[:, ql:ql + 128], pt[:, ql:ql + 128],
                                     causal01[:])
            pts[kc] = pt

        def pv(s, h, kc, kcmax, yp, pts):
            j = h // 4
            ql = max(0, kc * 128 - s * 512)
            lhsT = vA[kc][:, 0:2, :] if j == 0 else vA[kc][:, 2:4, :]
            nc.tensor.matmul(
                yp[:, ql:512], lhsT, pts[kc][:, ql:512],
                start=(kc == 0), stop=(kc == kcmax - 1),
                skip_group_check=True)
            pts[kc] = None

        def normalize(s, h, yp):
            # yp [128, 512]: rows 0:64 = softmax denominator l (64 copies),
            # rows 64:128 = unnormalized y (hd-major).
            p, r = h // 2, (h % 2) * 64
            # approx 1/l (~51 ULP), pipelined in two 256-col chunks to halve
            # the critical latency before the yp psum bank can be recycled.
            rls = p2t.tile([128, 512], F32, tag="rls", name="rls")
            for c0 in (0, 256):
                csl = slice(c0, c0 + 256)
                osl = slice(s * 512 + c0, s * 512 + c0 + 256)
                nc.vector.reciprocal_approx_fast(rls[0:64, csl],
                                                 yp[0:64, csl])
                nc.vector.tensor_mul(yT[p][r:r + 64, osl],
                                     yp[64:128, csl], rls[0:64, csl])

        def attention_half(s, fillers=()):
            # fillers: closures emitting independent PE work, spliced between
            # attention matmul groups so the PE never drains on softmax /
            # psum-recycle latency (keeps the HAM clock gate warm too).
            kcmax = 4 * (s + 1)
            fillers = list(fillers)
            fi = 0
            for hp in range(4):  # head pairs, 3-deep lookahead
                h0, h1 = 2 * hp, 2 * hp + 1
                yp0 = psB.tile([128, 512], F32, tag="yp", name="yp0")
                yp1 = psB.tile([128, 512], F32, tag="yp", name="yp1")
                pts0, pts1 = {}, {}
                for k in range(min(3, kcmax)):
                    scores(s, h0, k, pts0)
                    scores(s, h1, k, pts1)
                # spread filler work evenly across pair starts: the pair
                # boundary is where the PE stalls on yp recycle + softmax
                quota = -(-(len(fillers) - fi) // (4 - hp))  # ceil split
                for _ in range(quota):
                    fillers[fi]()
                    fi += 1
                for kc in range(kcmax):
                    if kc + 3 < kcmax:
                        scores(s, h0, kc + 3, pts0)
                        scores(s, h1, kc + 3, pts1)
                    pv(s, h0, kc, kcmax, yp0, pts0)
                    pv(s, h1, kc, kcmax, yp1, pts1)
                normalize(s, h0, yp0)
                normalize(s, h1, yp1)
            for f in fillers[fi:]:
                f()

        def mk_outproj_unit(tb, n):
            def go():
                trow = slice(tb * 128, (tb + 1) * 128)
                op = psA.tile([128, 512], F32, tag="sp", name="op")
                for p in range(4):
                    nc.tensor.matmul(
                        op[:], yT[p][:, trow],
                        wo[p][:, n * 512:(n + 1) * 512],
                        start=(p == 0), stop=(p == 3))
                oe = ost.tile([128, 512], BF16, tag="oe", name="oe")
                nc.vector.tensor_copy(oe[:], op[:])
                (nc.sync if n % 2 == 0 else nc.scalar).dma_start(
                    out_d[trow, n * 512:(n + 1) * 512], oe[:])
            return go

        th1_state = {}

        def mk_proj_blk(blk):
            def go():
                pp = psA.tile([128, 512], F32, tag="sp", name=f"pp1_{blk}")
                for ci in range(16):
                    w, cols = blk_w(ci, blk)
                    nc.tensor.matmul(pp[:], w[:, cols],
                                     xf[ci][:, 512:1024],
                                     start=(ci == 0), stop=(ci == 15))
                if blk == 5:
                    th1_state["vsb"] = v_evac(pp)
                else:
                    rope_blk(pp, blk, 1)
            return go

        def mk_vt(qb):
            def go():
                v_transpose_one(1, th1_state["vsb"], qb, p1pv)
            return go

        # s=0 attention with th1 projection blocks spliced in as PE filler;
        # the 4 v transposes are separate fillers so each one's psum tile
        # (1 buf) gets attention MMs to cover the evac copy latency.
        # Order balances filler TIME per pair-start (quota splits by count:
        # 3,3,2,2): each later pair still gets a ~3.5us proj block, not just
        # ~0.3us transposes — pairs 2-3 otherwise starve the PE on exp lag.
        attention_half(0, [mk_proj_blk(5), mk_proj_blk(4), mk_proj_blk(0),
                           mk_proj_blk(1), mk_vt(0), mk_vt(1),
                           mk_proj_blk(2), mk_vt(2),
                           mk_proj_blk(3), mk_vt(3)])

        p1x_cm.__exit__(None, None, None)
        p1t_cm.__exit__(None, None, None)
        p1w_cm.__exit__(None, None, None)

        p2c_cm = tc.tile_pool(name="p2c", bufs=1, side="right")
        p2c = p2c_cm.__enter__()
        ost_cm = tc.tile_pool(name="ost", bufs=4, side="right")
        ost = ost_cm.__enter__()
        wo = []
        for p in range(4):
            t = p2c.tile([128, C], BF16, tag=f"wo{p}", name=f"wo{p}")
            nc.scalar.dma_start(t[:], wo_d[p * 128:(p + 1) * 128, :])
            wo.append(t)

        # s=1 attention with s=0 out-proj units spliced in as PE filler
        attention_half(1, [mk_outproj_unit(tb, n)
                           for tb in range(4) for n in range(4)])
        for tb in range(4, 8):
            for n in range(4):
                mk_outproj_unit(tb, n)()

        for cm in (ost_cm, p2c_cm, psB_cm, psA_cm,
                   p2t_cm, ptp_cm, p1pv_cm, cp_cm):
            cm.__exit__(None, None, None)

    nc.compile()
    return nc


def make_core_inputs(x, cos, sin, attention_mask, is_vision, Wq, Wk, Wv, Wo,
                     gate, b, g):
    cos_b = np.asarray(cos[b], dtype=np.float32)   # [T, 64]
    sin_b = np.asarray(sin[b], dtype=np.float32)
    sgn = np.concatenate([-np.ones(32), np.ones(32)]).astype(np.float32)
    cosT = np.tile(cos_b.T, (2, 1))                            # [128, T]
    sinT = np.tile(sin_b.T * sgn[:, None], (2, 1))             # [128, T]
    vk = np.asarray(is_vision[b], dtype=np.int32)
    maskneg = np.where(np.asarray(attention_mask[b]) > 0, 0.0, NEG)

    hq0 = QH * g
    btab = np.empty((128, 128), dtype=np.float32)
    for h in range(QH):
        for vq in range(2):
            for kc in range(8):
                col = h * 16 + vq * 8 + kc
                ks = slice(kc * 128, (kc + 1) * 128)
                btab[:, col] = gate[hq0 + h, 2 * vq + vk[ks]] + maskneg[ks]

    return {
        "xT": np.ascontiguousarray(x[b].T).astype(NP_BF16),
        "wqT": np.ascontiguousarray(
            Wq[hq0 * 64:hq0 * 64 + 512, :].T).astype(NP_BF16),
        "wkvT": np.ascontiguousarray(
            np.concatenate([Wk[128 * g:128 * g + 128, :].T,
                            Wv[128 * g:128 * g + 128, :].T],
                           axis=1)).astype(NP_BF16),
        "woT": np.ascontiguousarray(
            Wo[:, hq0 * 64:hq0 * 64 + 512].T).astype(NP_BF16),
        "cosqT": np.ascontiguousarray(cosT * 0.125).astype(NP_BF16),
        "sinqT": np.ascontiguousarray(sinT * 0.125).astype(NP_BF16),
        "coskT": np.ascontiguousarray(cosT).astype(NP_BF16),
        "sinkT": np.ascontiguousarray(sinT).astype(NP_BF16),
        "btab": btab,
    }


def kernel(x, cos, sin, attention_mask, is_vision, Wq, Wk, Wv, Wo, gate):
    x = np.asarray(x, dtype=np.float32)
    cos = np.asarray(cos, dtype=np.float32)
    sin = np.asarray(sin, dtype=np.float32)
    attention_mask = np.asarray(attention_mask, dtype=np.float32)
    is_vision = np.asarray(is_vision)
    Wq = np.asarray(Wq, dtype=np.float32)
    Wk = np.asarray(Wk, dtype=np.float32)
    Wv = np.asarray(Wv, dtype=np.float32)
    Wo = np.asarray(Wo, dtype=np.float32)
    gate = np.asarray(gate, dtype=np.float32)

    # q-side vision flag must be constant within each 128-token tile and
    # identical across batches (holds for the fixed vision-prefix data).
    iv = is_vision.astype(np.int32)
    qtile_vq = []
    for qt in range(NTB):
        blk = iv[:, qt * 128:(qt + 1) * 128]
        assert (blk == blk[0, 0]).all(), "is_vision not 128-tile constant"
        qtile_vq.append(int(blk[0, 0]))

    in_maps = [
        make_core_inputs(x, cos, sin, attention_mask, is_vision,
                         Wq, Wk, Wv, Wo, gate, b=c // 4, g=c % 4)
        for c in range(NCORES)
    ]

    nc = build_program(qtile_vq)
    trace = bool(int(os.environ.get("NANOVLM_TRACE", "0")))
    if trace:
        results = _run_traced(nc, in_maps)
    else:
        results = run_bass_kernel_spmd(nc, in_maps, list(range(NCORES))).results
    out = np.empty((B, T, C), dtype=np.float32)
    for b in range(B):
        out[b] = sum(np.asarray(results[4 * b + g]["out"], dtype=np.float32)
                     for g in range(4))
    return out


def _ensure_ntff_hook():
    """The agent image's antenv lacks axon_hooks; shim it and register the
    ctypes NTFF profile hook against the axon PJRT .so."""
    try:
        from antenv.axon_hooks import get_axon_ntff_profile_hook  # noqa: F401
        return True
    except ImportError:
        pass
    import types

    import antenv

    mod = types.ModuleType("antenv.axon_hooks")
    mod._hook = None

    def set_axon_ntff_profile_hook(h):
        mod._hook = h

    def get_axon_ntff_profile_hook():
        return mod._hook

    mod.set_axon_ntff_profile_hook = set_axon_ntff_profile_hook
    mod.get_axon_ntff_profile_hook = get_axon_ntff_profile_hook
    sys.modules["antenv.axon_hooks"] = mod
    antenv.axon_hooks = mod
    if "/root/.axon_site" not in sys.path:
        sys.path.insert(0, "/root/.axon_site")
    try:
        from trn_agent_boot.trn_boot import _ntff_profile_via_ctypes

        hook = _ntff_profile_via_ctypes("/opt/axon/libaxon_pjrt.so")
    except Exception as e:
        print("ntff hook setup failed:", e)
        return False
    if hook is None:
        return False
    set_axon_ntff_profile_hook(hook)
    return True


def _run_traced(nc, in_maps, trace_core=0):
    import glob
    import tempfile

    from concourse import bass2jax
    from concourse._compat import FishPath
    import gauge.profiler

    if not _ensure_ntff_hook():
        print("no NTFF hook; running untraced")
        return run_bass_kernel_spmd(nc, in_maps, list(range(NCORES))).results

    from antenv.axon_hooks import get_axon_ntff_profile_hook

    hook = get_axon_ntff_profile_hook()
    tmpdir = tempfile.mkdtemp(prefix="nanovlm_prof_")
    with hook(tmpdir, [trace_core]):
        results = bass2jax.run_bass_via_pjrt(nc, in_maps, n_cores=NCORES)
    ntffs = glob.glob(os.path.join(tmpdir, "*_body*.ntff"))
    if not ntffs:
        print("no NTFF produced; files:", os.listdir(tmpdir))
        return results
    profile = gauge.profiler.Profile(
        profile_path=FishPath(tmpdir),
        kernel_dev_mode=True,
        profile_on_exit=False,
        bass_kernel=nc.m,
        offline_processing=True,
        fname="*_body*",
    )
    try:
        pr = profile.to_perfetto(model_index=(trace_core,))
        kernel.last_exec_time_ns = pr[0].exec_time_ns
        kernel.last_trace = pr[0].trace_path
        print(f"HW exec time: {pr[0].exec_time_ns} ns")
        print("trace:", pr[0].trace_path)
    except Exception as e:
        print("perfetto conversion failed:", type(e).__name__, e)
        print("ntff dir:", tmpdir)
    return results

